# revision 12
# baseline (speedup 1.0000x reference)

"""Causal attention (no head split) on 8 trn2 NeuronCores.

Reference computation (per batch b):
    q = x @ Wq^T ; k = x @ Wk^T ; v = x @ Wv^T          (nn.Linear convention)
    wei = softmax(mask(q @ k^T / sqrt(C)))               (causal)
    out = wei @ v

Algebraic restructuring (K and V are never materialized):
    S   = q k^T = x (Wq^T Wk) x^T = x M x^T     with M precomputed on host
    out = wei v = (wei x) Wv^T, i.e. O^T = Wv (x^T wei^T) = Wv H
so the device only computes:
    G^T = M^T xq^T                  (projection of this core's queries)
    S^T[s,t] = x^T(lhsT) G^T(rhs)   (contract over C)
    P^T = exp(S^T / 32) * mask      (per 128-row kv block)
    rowsum[t] += ones^T P^T         (PSUM-accumulated per pair)
    H[c,t] += x(lhsT) P^T(rhs)      (PSUM-accumulated across kv blocks)
    O^T = Wv^T-projection of H      (once per 2 pairs)
Final softmax normalization (divide by rowsum) happens on the host.

Sharding: 2 cores per batch (B=4), BALANCED at 128-row granularity.
The 16 query blocks of 128 rows have causal cost b+1 kv128-blocks each
(1..16).  Role A owns blocks {0,3,4,7,8,11,12,15} (cost 68), role B
{1,2,5,6,9,10,13,14} (cost 68).  Program position p (sorted by cost)
processes one 128-row query strip against kv blocks [0, 2p+2); the two
roles differ only in which global block sits at position p and in the
mask data applied to the last two kv blocks (diag-tri / full / zero
patterns, host-provided per role).  One SPMD instruction stream; all
role differences are input data.

Positions are processed in PAIRS (2j, 2j+1), j=0..3: kv blocks
[0, 4j+2) issue N=256 matmuls over the pair's adjacent gT/H columns;
the pair-younger position's extra kv blocks {4j+2, 4j+3} issue N=128.
H accumulates directly in PSUM across all of a pair's kv blocks
([128, 8, 256] f32 = 4 banks) and drains once per pair, so the vector
engine does no per-block adds.  The O projection runs once per 2 pairs
with N=512 chains.
"""
import numpy as np

import concourse.bass as bass
from concourse import bacc
import concourse.mybir as mybir
from concourse.tile import TileContext
from concourse import bass_utils

B, T, C = 4, 2048, 1024
P = 128
CS = C // P          # 8 contraction subtiles
NB = T // P          # 16 kv blocks of 128
NPOS = 8             # query positions (128-row strips) per core
PW = 256             # pair width (2 positions)
SCALE = 1.0 / np.sqrt(C)  # 1/32

F16 = mybir.dt.float16
F32 = mybir.dt.float32


def build():
    nc = bacc.Bacc(trn_type="TRN2", name="causal_attn")
    # host-packed layouts: every DMA has >=2KB contiguous DRAM runs
    xT = nc.dram_tensor("xT", [NB, P, CS * P], F16, kind="ExternalInput")
    xq = nc.dram_tensor("xq", [NPOS, P, CS * P], F16, kind="ExternalInput")
    xn = nc.dram_tensor("xn", [T, C], F16, kind="ExternalInput")
    wm = nc.dram_tensor("wm", [CS, P, C], F16, kind="ExternalInput")
    wv = nc.dram_tensor("wv", [CS, P, C], F16, kind="ExternalInput")
    masks = nc.dram_tensor("masks", [P, 4, P], F16, kind="ExternalInput")
    outp = nc.dram_tensor("outp", [2, P, CS * 512], F16, kind="ExternalOutput")
    rows = nc.dram_tensor("rows", [1, NPOS * P], F32, kind="ExternalOutput")

    xT_r = xT.rearrange("m p (cs t) -> m p cs t", t=P)
    xq_r = xq.rearrange("m p (cs t) -> m p cs t", t=P)
    xn_r = xn.rearrange("(m p) c -> p m c", p=P)
    wm_r = wm.rearrange("ds p (cs d) -> ds p cs d", d=P)
    wv_r = wv.rearrange("ds p (cs d) -> ds p cs d", d=P)
    out_r = outp.rearrange("j p (ds q) -> j p ds q", q=512)

    with TileContext(nc) as tc:
        with tc.tile_pool(name="keep", bufs=1) as keep, \
             tc.tile_pool(name="ppool", bufs=3) as ppool, \
             tc.tile_pool(name="hhpool", bufs=2) as hhpool, \
             tc.tile_pool(name="ostpool", bufs=2) as ostpool, \
             tc.tile_pool(name="psS", bufs=2, space="PSUM") as psS, \
             tc.tile_pool(name="psO", bufs=2, space="PSUM") as psO, \
             tc.tile_pool(name="psH", bufs=1, space="PSUM") as psH:

            xTs = keep.tile([P, CS, T], F16, tag="xTs")        # 32KB/part
            xqs = keep.tile([P, CS, NPOS * P], F16, tag="xqs")  # 16KB
            xns = keep.tile([P, NB, C], F16, tag="xns")        # 32KB
            wms = keep.tile([P, CS, C], F16, tag="wms")        # 16KB
            wvs = keep.tile([P, CS, C], F16, tag="wvs")        # 16KB
            gT = keep.tile([P, CS, NPOS * P], F16, tag="gT")   # 16KB
            msk = keep.tile([P, 4, P], F16, tag="msk")
            ones_t = keep.tile([P, 1], F16, tag="ones")
            rowsum = keep.tile([1, NPOS * P], F32, tag="rowsum")
            warm = keep.tile([P, 256], F16, tag="warm")

            # ---- Warm-up: spin the PE on dummy matmuls so the clock gate
            # ramps while the preamble + first input DMAs land.
            nc.vector.memset(warm[:], 0.0)
            nc.vector.memset(ones_t[:], 1.0)
            pw_ps = psO.tile([P, 512], F32, tag="po")
            for _ in range(96):
                nc.tensor.matmul(pw_ps[:, 0:64], warm[:, 0:P], warm[:, 0:64],
                                 start=True, stop=True)

            # ---- DMA schedule.  All loads go on the sync HWDGE ring; FIFO
            # issue order below IS the arrival priority.
            def ld_xT(m):
                nc.sync.dma_start(xTs[:, :, m * P:(m + 1) * P], xT_r[m])

            def ld_xq(m):
                nc.sync.dma_start(xqs[:, :, m * P:(m + 1) * P], xq_r[m])

            def ld_wm(ds):
                nc.sync.dma_start(wms[:, :, ds * P:(ds + 1) * P], wm_r[ds])

            def ld_xn(m):
                nc.sync.dma_start(xns[:, m], xn_r[:, m])

            # first transfers split in halves for more DMA-ring parallelism
            nc.sync.dma_start(wms[:, 0:4, 0:P], wm_r[0, :, 0:4])
            nc.sync.dma_start(wms[:, 4:8, 0:P], wm_r[0, :, 4:8])
            nc.sync.dma_start(xqs[:, 0:4, 0:P], xq_r[0, :, 0:4])
            nc.sync.dma_start(xqs[:, 4:8, 0:P], xq_r[0, :, 4:8])
            nc.sync.dma_start(xqs[:, 0:4, P:2 * P], xq_r[1, :, 0:4])
            nc.sync.dma_start(xqs[:, 4:8, P:2 * P], xq_r[1, :, 4:8])
            ld_xq(2); ld_xq(3)
            ld_wm(1); ld_wm(2); ld_wm(3); ld_wm(4)
            ld_wm(5); ld_wm(6); ld_wm(7)
            nc.sync.dma_start(msk[:], masks[:])
            for m in range(4):
                ld_xT(m)
            for m in range(4):
                ld_xn(m)
            ld_xq(4); ld_xq(5); ld_xq(6); ld_xq(7)
            for m in range(4, 8):
                ld_xT(m)
            for m in range(4, 8):
                ld_xn(m)
            for ds in range(CS):
                nc.sync.dma_start(wvs[:, :, ds * P:(ds + 1) * P], wv_r[ds])
            for m in range(8, 16):
                ld_xT(m)
            for m in range(8, 16):
                ld_xn(m)

            # ---- Phase G: G^T = M^T xq^T, N=512 chains (2 pairs at once).
            # (jq0, ds0) is split into N=256 halves so work starts as soon
            # as wm0 + the first two query strips land.
            for jq in range(2):
                for ds in range(CS):
                    dsl = slice(ds * P, (ds + 1) * P)
                    if jq == 0 and ds == 0:
                        for h in range(2):
                            pq = psO.tile([P, 256], F32, tag="po")
                            for cs in range(CS):
                                nc.tensor.matmul(
                                    pq[:], wms[:, cs, dsl],
                                    xqs[:, cs, h * 256:(h + 1) * 256],
                                    start=(cs == 0), stop=(cs == CS - 1))
                            nc.scalar.copy(gT[:, ds, h * 256:(h + 1) * 256], pq[:])
                        continue
                    pq = psO.tile([P, 512], F32, tag="po")
                    for cs in range(CS):
                        nc.tensor.matmul(
                            pq[:], wms[:, cs, dsl],
                            xqs[:, cs, jq * 512:(jq + 1) * 512],
                            start=(cs == 0), stop=(cs == CS - 1))
                    nc.scalar.copy(gT[:, ds, jq * 512:(jq + 1) * 512], pq[:])

            # ---- Pair loop.  Pair j: positions (2j, 2j+1), gT/H columns
            # [256j, 256j+256).  kv blocks m < 4j+2 wide (N=256), blocks
            # 4j+2 / 4j+3 narrow (N=128, younger position only).  Masks on
            # the elder position's last two kv blocks (cols 0:128 of wide)
            # and on both narrow blocks.
            for j in range(4):
                mw = 4 * j + 2          # number of wide kv blocks
                qsl = slice(j * PW, (j + 1) * PW)
                nsl = slice(j * PW + P, (j + 1) * PW)
                if j % 2 == 0:  # one hh tile spans 2 pairs
                    hh = hhpool.tile([P, CS, 512], F16, tag="hh")
                hcol = (j % 2) * PW
                ph = psH.tile([P, CS, PW], F32, tag="ph")
                for m in range(mw + 2):
                    wide = m < mw
                    W = PW if wide else P
                    # st occupies the bank's first half; the second half
                    # hosts this slot's transient rowsum partial (rwp)
                    stt = psS.tile([P, 512], F32, tag="st")
                    st = stt[:, 0:W]
                    rwp = stt[0:1, PW:PW + W]
                    for cs in range(CS):
                        nc.tensor.matmul(
                            st[:], xTs[:, cs, m * P:(m + 1) * P],
                            gT[:, cs, qsl if wide else nsl],
                            start=(cs == 0), stop=(cs == CS - 1))
                    pT = ppool.tile([P, W], F16, tag="pT")
                    nc.scalar.activation(
                        pT[:], st[:],
                        mybir.ActivationFunctionType.Exp, scale=float(SCALE))
                    # masks: elder position's diag pair on wide cols 0:128,
                    # younger's on the narrow blocks (role-carried patterns)
                    if m == mw - 2:
                        nc.vector.tensor_mul(pT[:, 0:P], pT[:, 0:P], msk[:, 0])
                    elif m == mw - 1:
                        nc.vector.tensor_mul(pT[:, 0:P], pT[:, 0:P], msk[:, 1])
                    elif m == mw:
                        nc.vector.tensor_mul(pT[:], pT[:], msk[:, 2])
                    elif m == mw + 1:
                        nc.vector.tensor_mul(pT[:], pT[:], msk[:, 3])

                    # per-slot rowsum partial (transient), accumulated into
                    # SBUF by the vector engine
                    rsl = slice(j * PW, (j + 1) * PW) if wide else \
                        slice(j * PW + P, (j + 1) * PW)
                    nc.tensor.matmul(rwp, ones_t[:], pT[:],
                                     start=True, stop=True)
                    if m == 0:
                        nc.vector.tensor_copy(rowsum[:, rsl], rwp)
                    else:
                        nc.vector.tensor_add(rowsum[:, rsl], rowsum[:, rsl],
                                             rwp)

                    # H[c, pair cols] += x(lhsT) @ P^T, accumulated in PSUM.
                    # PSUM start_tensor_calc marks pending-zero at 2KB
                    # zero-region granularity; each ph[:, cs] is 1KB, so only
                    # even cs carries start=True — the rounded marking covers
                    # the odd sibling, whose m=0 write then auto-zeroes via
                    # the pending flag.
                    for cs in range(CS):
                        xl = xns[:, m, cs * P:(cs + 1) * P]
                        if wide and m < mw - 1:
                            nc.tensor.matmul(ph[:, cs], xl, pT[:],
                                             start=(m == 0 and cs % 2 == 0),
                                             stop=False)
                        elif m == mw - 1:
                            nc.tensor.matmul(ph[:, cs, 0:P], xl, pT[:, 0:P],
                                             start=False, stop=True)
                            nc.tensor.matmul(ph[:, cs, P:PW], xl, pT[:, P:PW],
                                             start=False, stop=False)
                        else:
                            nc.tensor.matmul(ph[:, cs, P:PW], xl, pT[:],
                                             start=False, stop=(m == mw + 1))

                nc.sync.dma_start(rows[:, j * PW:(j + 1) * PW],
                                  rowsum[:, j * PW:(j + 1) * PW])
                # drain the pair's H into SBUF (alternate copy engines)
                for cs in range(CS):
                    if cs % 2 == 0:
                        nc.scalar.copy(hh[:, cs, hcol:hcol + PW], ph[:, cs])
                    else:
                        nc.vector.tensor_copy(hh[:, cs, hcol:hcol + PW],
                                              ph[:, cs])

                # ---- O projection once per 2 pairs: O^T = Wv^T H, N=512.
                if j % 2 == 1:
                    jq = j // 2
                    ost = ostpool.tile([P, CS, 512], F16, tag="ost")
                    for ds in range(CS):
                        pf = psO.tile([P, 512], F32, tag="po")
                        for cs in range(CS):
                            nc.tensor.matmul(
                                pf[:], wvs[:, cs, ds * P:(ds + 1) * P],
                                hh[:, cs], start=(cs == 0), stop=(cs == CS - 1))
                        # alternate copy engines; smaller trailing stores on
                        # the final chain so the last DMA drains quickly
                        if ds % 2 == 0:
                            nc.scalar.copy(ost[:, ds], pf[:])
                            if jq == 1 and ds == 6:
                                nc.sync.dma_start(out_r[jq, :, 6:7], ost[:, 6:7])
                        else:
                            if jq == 1 and ds == 7:
                                nc.scalar.copy(ost[:, ds, 0:256], pf[:, 0:256])
                                nc.vector.tensor_copy(ost[:, ds, 256:512],
                                                      pf[:, 256:512])
                                nc.sync.dma_start(out_r[jq, :, 7:8], ost[:, 7:8])
                            else:
                                nc.vector.tensor_copy(ost[:, ds], pf[:])
                                nc.sync.dma_start(out_r[jq, :, ds - 1:ds + 1],
                                                  ost[:, ds - 1:ds + 1])

    nc.compile()
    return nc


_NC = None


def _get_nc():
    global _NC
    if _NC is None:
        _NC = build()
    return _NC


# per-role query 128-blocks in program-position order (balanced causal
# cost: both rosters sum to 68 kv128-block units)
_ORDER_A = np.array([0, 3, 4, 7, 8, 11, 12, 15])
_ORDER_B = np.array([1, 2, 5, 6, 9, 10, 13, 14])


def _col_to_row(order):
    """Map device output column u (pair-major) -> global query row."""
    u = np.arange(NPOS * P)
    pair, q = u // PW, u % PW
    pos = 2 * pair + (q >= P).astype(int)
    return order[pos] * P + (q % P)


def make_in_maps(x, Wq, Wk, Wv):
    x = np.asarray(x, dtype=np.float32)
    wq64 = np.asarray(Wq, np.float64)
    wk64 = np.asarray(Wk, np.float64)
    M = (wq64.T @ wk64).astype(np.float16)                   # [c, d]
    WvT = np.asarray(Wv, np.float32).T.astype(np.float16)    # [c, d]
    # pack [c, d] -> [ds, p, cs*128+d2] so per-partition runs are 2KB
    def packw(w):
        return np.ascontiguousarray(
            w.reshape(CS, P, CS, P).transpose(2, 1, 0, 3).reshape(CS, P, C))
    wmp, wvp = packw(M), packw(WvT)

    tri = (np.arange(P)[:, None] <= np.arange(P)[None, :]).astype(np.float16)
    zer = np.zeros((P, P), np.float16)
    one = np.ones((P, P), np.float16)
    # mask[k] applies to kv block (mw-2+k) of each pair; see build()
    mask_A = np.ascontiguousarray(
        np.stack([tri, zer, one, tri], axis=0).transpose(1, 0, 2))
    mask_B = np.ascontiguousarray(
        np.stack([one, tri, tri, zer], axis=0).transpose(1, 0, 2))

    in_maps = []
    for core in range(8):
        b, role = divmod(core, 2)
        order = _ORDER_A if role == 0 else _ORDER_B
        xh = x[b].astype(np.float16)                         # [T, C] global
        xn_in = np.ascontiguousarray(xh)
        xTp = np.ascontiguousarray(
            xh.reshape(NB, P, CS, P).transpose(0, 3, 2, 1)   # [m,p,cs,t2]
            .reshape(NB, P, CS * P))
        xqh = xh.reshape(NB, P, C)[order]                    # [pos,128,C]
        xqp = np.ascontiguousarray(
            xqh.reshape(NPOS, P, CS, P).transpose(0, 3, 2, 1)
            .reshape(NPOS, P, CS * P))
        in_maps.append({
            "xT": xTp,
            "xq": xqp,
            "xn": xn_in,
            "wm": wmp, "wv": wvp,
            "masks": mask_A if role == 0 else mask_B,
        })
    return in_maps


def assemble(results):
    out = np.empty((B, T, C), np.float32)
    rowmap_A = _col_to_row(_ORDER_A)
    rowmap_B = _col_to_row(_ORDER_B)
    for core in range(8):
        b, role = divmod(core, 2)
        rowmap = rowmap_A if role == 0 else rowmap_B
        op = results[core]["outp"].astype(np.float32)
        oT = op.reshape(2, P, CS, 512).transpose(2, 1, 0, 3).reshape(C, NPOS * P)
        rsum = results[core]["rows"].reshape(NPOS * P)
        o = oT.T / rsum[:, None]                             # [1024 q, C]
        out[b, rowmap] = o
    return out


def kernel(x, Wq, Wk, Wv):
    nc = _get_nc()
    in_maps = make_in_maps(x, Wq, Wk, Wv)
    res = bass_utils.run_bass_kernel_spmd(nc, in_maps, core_ids=list(range(8)))
    return assemble(res.results)


def _install_trace_shim():
    """Provide antenv.axon_hooks (absent in this image) so trace=True works."""
    import sys
    import types
    if "antenv.axon_hooks" in sys.modules:
        return
    hook_box = [None]
    mod = types.ModuleType("antenv.axon_hooks")
    mod.set_axon_ntff_profile_hook = lambda h: hook_box.__setitem__(0, h)
    mod.get_axon_ntff_profile_hook = lambda: hook_box[0]
    import antenv
    sys.modules["antenv.axon_hooks"] = mod
    antenv.axon_hooks = mod
    try:
        from trn_agent_boot.trn_boot import _ntff_profile_via_ctypes
        mod.set_axon_ntff_profile_hook(
            _ntff_profile_via_ctypes("/opt/axon/libaxon_pjrt.so"))
    except Exception:
        pass


def run_traced(x, Wq, Wk, Wv):
    """Like kernel() but with NTFF tracing; returns (out, BassKernelResults)."""
    _install_trace_shim()
    nc = _get_nc()
    in_maps = make_in_maps(x, Wq, Wk, Wv)
    res = bass_utils.run_bass_kernel_spmd(
        nc, in_maps, core_ids=list(range(8)), trace=True,
        trace_cores=list(range(8)))
    return assemble(res.results), res


# revision 19
# speedup vs baseline: 1.0134x; 1.0134x over previous

"""Causal attention (no head split) on 8 trn2 NeuronCores.

Reference computation (per batch b):
    q = x @ Wq^T ; k = x @ Wk^T ; v = x @ Wv^T          (nn.Linear convention)
    wei = softmax(mask(q @ k^T / sqrt(C)))               (causal)
    out = wei @ v

Algebraic restructuring (K and V are never materialized):
    S   = q k^T = x (Wq^T Wk) x^T = x M x^T     with M precomputed on host
    out = wei v = (wei x) Wv^T, i.e. O^T = Wv (x^T wei^T) = Wv H
so the device only computes:
    G^T = M^T xq^T                  (projection of this core's queries)
    S^T[s,t] = x^T(lhsT) G^T(rhs)   (contract over C)
    P^T = exp(S^T / 32) * mask      (per 128-row kv block)
    rowsum[t] += ones^T P^T         (PSUM-accumulated per pair)
    H[c,t] += x(lhsT) P^T(rhs)      (PSUM-accumulated across kv blocks)
    O^T = Wv^T-projection of H      (once per 2 pairs)
Final softmax normalization (divide by rowsum) happens on the host.

Sharding: 2 cores per batch (B=4), BALANCED at 128-row granularity.
The 16 query blocks of 128 rows have causal cost b+1 kv128-blocks each
(1..16).  Role A owns blocks {0,3,4,7,8,11,12,15} (cost 68), role B
{1,2,5,6,9,10,13,14} (cost 68).  Program position p (sorted by cost)
processes one 128-row query strip against kv blocks [0, 2p+2); the two
roles differ only in which global block sits at position p and in the
mask data applied to the last two kv blocks (diag-tri / full / zero
patterns, host-provided per role).  One SPMD instruction stream; all
role differences are input data.

Positions are processed in PAIRS (2j, 2j+1), j=0..3: kv blocks
[0, 4j+2) issue N=256 matmuls over the pair's adjacent gT/H columns;
the pair-younger position's extra kv blocks {4j+2, 4j+3} issue N=128.
H accumulates directly in PSUM across all of a pair's kv blocks
([128, 8, 256] f32 = 4 banks) and drains once per pair, so the vector
engine does no per-block adds.  The O projection runs once per 2 pairs
with N=512 chains.
"""
import numpy as np

import concourse.bass as bass
from concourse import bacc
import concourse.mybir as mybir
from concourse.tile import TileContext
from concourse import bass_utils

B, T, C = 4, 2048, 1024
P = 128
CS = C // P          # 8 contraction subtiles
NB = T // P          # 16 kv blocks of 128
NPOS = 8             # query positions (128-row strips) per core
PW = 256             # pair width (2 positions)
SCALE = 1.0 / np.sqrt(C)  # 1/32

F16 = mybir.dt.float16
F32 = mybir.dt.float32


def build():
    nc = bacc.Bacc(trn_type="TRN2", name="causal_attn")
    # host-packed layouts: every DMA has >=2KB contiguous DRAM runs
    xT = nc.dram_tensor("xT", [NB, P, CS * P], F16, kind="ExternalInput")
    xq = nc.dram_tensor("xq", [NPOS, P, CS * P], F16, kind="ExternalInput")
    xn = nc.dram_tensor("xn", [T, C], F16, kind="ExternalInput")
    wm = nc.dram_tensor("wm", [CS, P, C], F16, kind="ExternalInput")
    wv = nc.dram_tensor("wv", [CS, P, C], F16, kind="ExternalInput")
    masks = nc.dram_tensor("masks", [P, 4, P], F16, kind="ExternalInput")
    outp = nc.dram_tensor("outp", [2, P, CS * 512], F16, kind="ExternalOutput")
    rows = nc.dram_tensor("rows", [1, NPOS * P], F32, kind="ExternalOutput")

    xT_r = xT.rearrange("m p (cs t) -> m p cs t", t=P)
    xq_r = xq.rearrange("m p (cs t) -> m p cs t", t=P)
    xn_r = xn.rearrange("(m p) c -> p m c", p=P)
    wm_r = wm.rearrange("ds p (cs d) -> ds p cs d", d=P)
    wv_r = wv.rearrange("ds p (cs d) -> ds p cs d", d=P)
    out_r = outp.rearrange("j p (ds q) -> j p ds q", q=512)
    out_rf = outp.rearrange("j p (dq q) -> j p dq q", q=128)

    with TileContext(nc) as tc:
        with tc.tile_pool(name="keep", bufs=1) as keep, \
             tc.tile_pool(name="ppool", bufs=3) as ppool, \
             tc.tile_pool(name="hhpool", bufs=2) as hhpool, \
             tc.tile_pool(name="ostpool", bufs=2) as ostpool, \
             tc.tile_pool(name="psS", bufs=2, space="PSUM") as psS, \
             tc.tile_pool(name="psO", bufs=2, space="PSUM") as psO, \
             tc.tile_pool(name="psH", bufs=1, space="PSUM") as psH:

            xTs = keep.tile([P, CS, T], F16, tag="xTs")        # 32KB/part
            xqs = keep.tile([P, CS, NPOS * P], F16, tag="xqs")  # 16KB
            xns = keep.tile([P, NB, C], F16, tag="xns")        # 32KB
            wms = keep.tile([P, CS, C], F16, tag="wms")        # 16KB
            wvs = keep.tile([P, CS, C], F16, tag="wvs")        # 16KB
            gT = keep.tile([P, CS, NPOS * P], F16, tag="gT")   # 16KB
            msk = keep.tile([P, 4, P], F16, tag="msk")
            ones_t = keep.tile([P, P], F16, tag="ones")
            rowsum = keep.tile([1, NPOS * P], F32, tag="rowsum")
            warm = keep.tile([P, 256], F16, tag="warm")

            # ---- Warm-up: spin the PE on dummy matmuls so the clock gate
            # ramps while the preamble + first input DMAs land.
            nc.vector.memset(warm[:], 0.0)
            nc.vector.memset(ones_t[:], 1.0)
            pw_ps = psO.tile([P, 512], F32, tag="po")
            for _ in range(84):
                nc.tensor.matmul(pw_ps[:, 0:64], warm[:, 0:P], warm[:, 0:64],
                                 start=True, stop=True)

            # ---- DMA schedule.  All loads go on the sync HWDGE ring; FIFO
            # issue order below IS the arrival priority.
            def ld_xT(m):
                nc.sync.dma_start(xTs[:, :, m * P:(m + 1) * P], xT_r[m])

            def ld_xq(m):
                nc.sync.dma_start(xqs[:, :, m * P:(m + 1) * P], xq_r[m])

            def ld_wm(ds):
                nc.sync.dma_start(wms[:, :, ds * P:(ds + 1) * P], wm_r[ds])

            def ld_xn(m):
                nc.sync.dma_start(xns[:, m], xn_r[:, m])

            # early transfers split in halves across DMA rings so the G
            # phase is not gated on single-ring transfer bandwidth
            def ld_wm2(ds):
                nc.sync.dma_start(wms[:, 0:4, ds * P:(ds + 1) * P],
                                  wm_r[ds, :, 0:4])
                nc.sync.dma_start(wms[:, 4:8, ds * P:(ds + 1) * P],
                                  wm_r[ds, :, 4:8])

            def ld_xq2(m):
                nc.sync.dma_start(xqs[:, 0:4, m * P:(m + 1) * P],
                                  xq_r[m, :, 0:4])
                nc.sync.dma_start(xqs[:, 4:8, m * P:(m + 1) * P],
                                  xq_r[m, :, 4:8])

            ld_wm2(0); ld_xq2(0); ld_xq2(1); ld_xq2(2); ld_xq2(3)
            ld_wm2(1); ld_wm2(2); ld_wm2(3); ld_wm2(4)
            ld_wm2(5); ld_wm2(6); ld_wm2(7)
            nc.sync.dma_start(msk[:], masks[:])
            for m in range(4):
                ld_xT(m)
            for m in range(4):
                ld_xn(m)
            ld_xq(4); ld_xq(5); ld_xq(6); ld_xq(7)
            for m in range(4, 8):
                ld_xT(m)
            for m in range(4, 8):
                ld_xn(m)
            for ds in range(CS):
                nc.sync.dma_start(wvs[:, :, ds * P:(ds + 1) * P], wv_r[ds])
            for m in range(8, 16):
                ld_xT(m)
            for m in range(8, 16):
                ld_xn(m)

            # ---- Phase G: G^T = M^T xq^T, N=512 chains (2 pairs at once).
            # (jq0, ds0) is split into N=256 halves so work starts as soon
            # as wm0 + the first two query strips land.
            for jq in range(2):
                for ds in range(CS):
                    dsl = slice(ds * P, (ds + 1) * P)
                    if jq == 0 and ds == 0:
                        for h in range(2):
                            pq = psO.tile([P, 256], F32, tag="po")
                            for cs in range(CS):
                                nc.tensor.matmul(
                                    pq[:], wms[:, cs, dsl],
                                    xqs[:, cs, h * 256:(h + 1) * 256],
                                    start=(cs == 0), stop=(cs == CS - 1))
                            nc.scalar.copy(gT[:, ds, h * 256:(h + 1) * 256], pq[:])
                        continue
                    pq = psO.tile([P, 512], F32, tag="po")
                    for cs in range(CS):
                        nc.tensor.matmul(
                            pq[:], wms[:, cs, dsl],
                            xqs[:, cs, jq * 512:(jq + 1) * 512],
                            start=(cs == 0), stop=(cs == CS - 1))
                    nc.scalar.copy(gT[:, ds, jq * 512:(jq + 1) * 512], pq[:])

            # ---- Pair loop.  Pair j: positions (2j, 2j+1), gT/H columns
            # [256j, 256j+256).  kv blocks m < 4j+2 wide (N=256), blocks
            # 4j+2 / 4j+3 narrow (N=128, younger position only).  Masks on
            # the elder position's last two kv blocks (cols 0:128 of wide)
            # and on both narrow blocks.
            for j in range(4):
                mw = 4 * j + 2          # number of wide kv blocks
                qsl = slice(j * PW, (j + 1) * PW)
                nsl = slice(j * PW + P, (j + 1) * PW)
                if j % 2 == 0:  # one hh tile spans 2 pairs
                    hh = hhpool.tile([P, CS, 512], F16, tag="hh")
                hcol = (j % 2) * PW
                ph = psH.tile([P, CS, PW], F32, tag="ph")
                for m in range(mw + 2):
                    wide = m < mw
                    W = PW if wide else P
                    # st occupies the bank's first half; the second half
                    # hosts this slot's transient rowsum partial (rwp)
                    stt = psS.tile([P, 512], F32, tag="st")
                    st = stt[:, 0:W]
                    rwp = stt[0:1, PW:PW + W]
                    for cs in range(CS):
                        nc.tensor.matmul(
                            st[:], xTs[:, cs, m * P:(m + 1) * P],
                            gT[:, cs, qsl if wide else nsl],
                            start=(cs == 0), stop=(cs == CS - 1))
                    pT = ppool.tile([P, W], F16, tag="pT")
                    nc.scalar.activation(
                        pT[:], st[:],
                        mybir.ActivationFunctionType.Exp, scale=float(SCALE))
                    # masks: elder position's diag pair on wide cols 0:128,
                    # younger's on the narrow blocks (role-carried patterns)
                    if m == mw - 2:
                        nc.vector.tensor_mul(pT[:, 0:P], pT[:, 0:P], msk[:, 0])
                    elif m == mw - 1:
                        nc.vector.tensor_mul(pT[:, 0:P], pT[:, 0:P], msk[:, 1])
                    elif m == mw:
                        nc.vector.tensor_mul(pT[:], pT[:], msk[:, 2])
                    elif m == mw + 1:
                        nc.vector.tensor_mul(pT[:], pT[:], msk[:, 3])

                    # per-slot rowsum partial (transient), accumulated into
                    # SBUF by the vector engine.  ones lhsT is [128, 128]
                    # (every output row = rowsum); M=1 matmuls pay a ~2x
                    # per-instruction floor, M=128 streams at N cycles.
                    rsl = slice(j * PW, (j + 1) * PW) if wide else \
                        slice(j * PW + P, (j + 1) * PW)
                    nc.tensor.matmul(stt[:, PW:PW + W], ones_t[:], pT[:],
                                     start=True, stop=True)
                    if m == 0:
                        nc.vector.tensor_copy(rowsum[:, rsl], rwp)
                    else:
                        nc.vector.tensor_add(rowsum[:, rsl], rowsum[:, rsl],
                                             rwp)

                    # H[c, pair cols] += x(lhsT) @ P^T, accumulated in PSUM.
                    # PSUM start_tensor_calc marks pending-zero at 2KB
                    # zero-region granularity; each ph[:, cs] is 1KB, so only
                    # even cs carries start=True — the rounded marking covers
                    # the odd sibling, whose m=0 write then auto-zeroes via
                    # the pending flag.
                    for cs in range(CS):
                        xl = xns[:, m, cs * P:(cs + 1) * P]
                        if wide and m < mw - 1:
                            nc.tensor.matmul(ph[:, cs], xl, pT[:],
                                             start=(m == 0 and cs % 2 == 0),
                                             stop=False)
                        elif m == mw - 1:
                            nc.tensor.matmul(ph[:, cs, 0:P], xl, pT[:, 0:P],
                                             start=False, stop=True)
                            nc.tensor.matmul(ph[:, cs, P:PW], xl, pT[:, P:PW],
                                             start=False, stop=False)
                        else:
                            nc.tensor.matmul(ph[:, cs, P:PW], xl, pT[:],
                                             start=False, stop=(m == mw + 1))

                nc.sync.dma_start(rows[:, j * PW:(j + 1) * PW],
                                  rowsum[:, j * PW:(j + 1) * PW])
                # drain the pair's H into SBUF (alternate copy engines)
                for cs in range(CS):
                    if cs % 2 == 0:
                        nc.scalar.copy(hh[:, cs, hcol:hcol + PW], ph[:, cs])
                    else:
                        nc.vector.tensor_copy(hh[:, cs, hcol:hcol + PW],
                                              ph[:, cs])

                # ---- O projection once per 2 pairs: O^T = Wv^T H, N=512.
                if j % 2 == 1:
                    jq = j // 2
                    ost = ostpool.tile([P, CS, 512], F16, tag="ost")
                    for ds in range(CS):
                        pf = psO.tile([P, 512], F32, tag="po")
                        for cs in range(CS):
                            nc.tensor.matmul(
                                pf[:], wvs[:, cs, ds * P:(ds + 1) * P],
                                hh[:, cs], start=(cs == 0), stop=(cs == CS - 1))
                        # alternate copy engines.  jq0 stores go as ds pairs
                        # (plenty of slack); jq1 stores stream out in small
                        # chunks across many DMA rings so the final store is
                        # not single-ring bound at kernel end.
                        ost_rf = ost.rearrange("p ds (c q) -> p (ds c) q", q=P)
                        if ds % 2 == 0:
                            nc.scalar.copy(ost[:, ds], pf[:])
                            if jq == 1:
                                nc.sync.dma_start(
                                    out_rf[jq, :, 4 * ds:4 * ds + 2],
                                    ost_rf[:, 4 * ds:4 * ds + 2])
                                nc.sync.dma_start(
                                    out_rf[jq, :, 4 * ds + 2:4 * ds + 4],
                                    ost_rf[:, 4 * ds + 2:4 * ds + 4])
                        else:
                            if jq == 1 and ds == 7:
                                nc.scalar.copy(ost[:, ds, 0:256], pf[:, 0:256])
                                for c in range(2):
                                    nc.sync.dma_start(
                                        out_rf[jq, :, 28 + c:29 + c],
                                        ost_rf[:, 28 + c:29 + c])
                                nc.vector.tensor_copy(ost[:, ds, 256:512],
                                                      pf[:, 256:512])
                                for c in range(2, 4):
                                    nc.sync.dma_start(
                                        out_rf[jq, :, 28 + c:29 + c],
                                        ost_rf[:, 28 + c:29 + c])
                            else:
                                nc.vector.tensor_copy(ost[:, ds], pf[:])
                                if jq == 0:
                                    nc.sync.dma_start(
                                        out_r[jq, :, ds - 1:ds + 1],
                                        ost[:, ds - 1:ds + 1])
                                else:
                                    nc.sync.dma_start(
                                        out_rf[jq, :, 4 * ds:4 * ds + 2],
                                        ost_rf[:, 4 * ds:4 * ds + 2])
                                    nc.sync.dma_start(
                                        out_rf[jq, :, 4 * ds + 2:4 * ds + 4],
                                        ost_rf[:, 4 * ds + 2:4 * ds + 4])

    nc.compile()
    return nc


_NC = None


def _get_nc():
    global _NC
    if _NC is None:
        _NC = build()
    return _NC


# per-role query 128-blocks in program-position order (balanced causal
# cost: both rosters sum to 68 kv128-block units)
_ORDER_A = np.array([0, 3, 4, 7, 8, 11, 12, 15])
_ORDER_B = np.array([1, 2, 5, 6, 9, 10, 13, 14])


def _col_to_row(order):
    """Map device output column u (pair-major) -> global query row."""
    u = np.arange(NPOS * P)
    pair, q = u // PW, u % PW
    pos = 2 * pair + (q >= P).astype(int)
    return order[pos] * P + (q % P)


def make_in_maps(x, Wq, Wk, Wv):
    x = np.asarray(x, dtype=np.float32)
    wq64 = np.asarray(Wq, np.float64)
    wk64 = np.asarray(Wk, np.float64)
    M = (wq64.T @ wk64).astype(np.float16)                   # [c, d]
    WvT = np.asarray(Wv, np.float32).T.astype(np.float16)    # [c, d]
    # pack [c, d] -> [ds, p, cs*128+d2] so per-partition runs are 2KB
    def packw(w):
        return np.ascontiguousarray(
            w.reshape(CS, P, CS, P).transpose(2, 1, 0, 3).reshape(CS, P, C))
    wmp, wvp = packw(M), packw(WvT)

    tri = (np.arange(P)[:, None] <= np.arange(P)[None, :]).astype(np.float16)
    zer = np.zeros((P, P), np.float16)
    one = np.ones((P, P), np.float16)
    # mask[k] applies to kv block (mw-2+k) of each pair; see build()
    mask_A = np.ascontiguousarray(
        np.stack([tri, zer, one, tri], axis=0).transpose(1, 0, 2))
    mask_B = np.ascontiguousarray(
        np.stack([one, tri, tri, zer], axis=0).transpose(1, 0, 2))

    in_maps = []
    for core in range(8):
        b, role = divmod(core, 2)
        order = _ORDER_A if role == 0 else _ORDER_B
        xh = x[b].astype(np.float16)                         # [T, C] global
        xn_in = np.ascontiguousarray(xh)
        xTp = np.ascontiguousarray(
            xh.reshape(NB, P, CS, P).transpose(0, 3, 2, 1)   # [m,p,cs,t2]
            .reshape(NB, P, CS * P))
        xqh = xh.reshape(NB, P, C)[order]                    # [pos,128,C]
        xqp = np.ascontiguousarray(
            xqh.reshape(NPOS, P, CS, P).transpose(0, 3, 2, 1)
            .reshape(NPOS, P, CS * P))
        in_maps.append({
            "xT": xTp,
            "xq": xqp,
            "xn": xn_in,
            "wm": wmp, "wv": wvp,
            "masks": mask_A if role == 0 else mask_B,
        })
    return in_maps


def assemble(results):
    out = np.empty((B, T, C), np.float32)
    rowmap_A = _col_to_row(_ORDER_A)
    rowmap_B = _col_to_row(_ORDER_B)
    for core in range(8):
        b, role = divmod(core, 2)
        rowmap = rowmap_A if role == 0 else rowmap_B
        op = results[core]["outp"].astype(np.float32)
        oT = op.reshape(2, P, CS, 512).transpose(2, 1, 0, 3).reshape(C, NPOS * P)
        rsum = results[core]["rows"].reshape(NPOS * P)
        o = oT.T / rsum[:, None]                             # [1024 q, C]
        out[b, rowmap] = o
    return out


def kernel(x, Wq, Wk, Wv):
    nc = _get_nc()
    in_maps = make_in_maps(x, Wq, Wk, Wv)
    res = bass_utils.run_bass_kernel_spmd(nc, in_maps, core_ids=list(range(8)))
    return assemble(res.results)


def _install_trace_shim():
    """Provide antenv.axon_hooks (absent in this image) so trace=True works."""
    import sys
    import types
    if "antenv.axon_hooks" in sys.modules:
        return
    hook_box = [None]
    mod = types.ModuleType("antenv.axon_hooks")
    mod.set_axon_ntff_profile_hook = lambda h: hook_box.__setitem__(0, h)
    mod.get_axon_ntff_profile_hook = lambda: hook_box[0]
    import antenv
    sys.modules["antenv.axon_hooks"] = mod
    antenv.axon_hooks = mod
    try:
        from trn_agent_boot.trn_boot import _ntff_profile_via_ctypes
        mod.set_axon_ntff_profile_hook(
            _ntff_profile_via_ctypes("/opt/axon/libaxon_pjrt.so"))
    except Exception:
        pass


def run_traced(x, Wq, Wk, Wv):
    """Like kernel() but with NTFF tracing; returns (out, BassKernelResults)."""
    _install_trace_shim()
    nc = _get_nc()
    in_maps = make_in_maps(x, Wq, Wk, Wv)
    res = bass_utils.run_bass_kernel_spmd(
        nc, in_maps, core_ids=list(range(8)), trace=True,
        trace_cores=list(range(8)))
    return assemble(res.results), res
